# revision 1
# baseline (speedup 1.0000x reference)
"""Trainium2 Bass kernel for the FCBlock weight-transform + matmul problem.

Math (per reference):
    W_i = per-head 3x3 conv over W.reshape(4, 1024, 4096) + conv_b
          + sigmoid(sk_wt) * W            (per-head scalars)
    out  = inp @ W_i.T                    (inp: [2, 2048, 4096])

Strategy: tensor-parallel shard of W along fout across 8 NeuronCores
(512 rows each; the host pre-slices W with 1-row conv halo, zero-padded
at head boundaries).  On each core:
  - build the 3x3 conv as tiny banded matrices (from conv_w/conv_b/sk_wt,
    broadcast on device) and run the weight transform as PE band-matmuls
    accumulating in PSUM (sigmoid-gated residual folded into the center
    diagonal; bias added during the PSUM->SBUF copy),
  - transpose W_i on the PE (fin onto partitions),
  - stream inp tiles (DMA f32->bf16 cast), PE-transpose them, and run the
    main matmul in bf16 with fp32 PSUM accumulation.
Output is sharded on fout; the host concatenates.
"""

import numpy as np

import concourse.bass as bass
import concourse.mybir as mybir
import concourse.tile as tile
from concourse import bacc
from concourse.bass_utils import run_bass_kernel_spmd
from concourse.masks import make_identity

F32 = mybir.dt.float32
BF16 = mybir.dt.bfloat16

NCORES = 8
NUM_HEADS = 4
TOK = 4096          # 2 * 2048 tokens
FIN = 4096
FOUT = 4096
FSH = FOUT // NCORES  # 512 fout rows per core


def build_program(tok=TOK, fin=FIN, repeat=1, probe=()):
    """Build the per-core SPMD program.

    tok/fin are parameters so a mini variant can be compiled quickly for
    validation; the graded path always uses the full sizes.
    """
    assert tok % 128 == 0 and fin % 512 == 0
    n_tblk = tok // 128          # 128-token blocks
    n_strip = fin // 512         # 512-col fin strips
    n_k = fin // 128             # 128-deep contraction blocks
    n_win = FSH // 128           # 4 fout row windows per core

    nc = bacc.Bacc(None, target_bir_lowering=False)

    inp = nc.declare_dram_parameter("inp", [tok, fin], F32, isOutput=False)
    wh = nc.declare_dram_parameter("wh", [FSH + 2, fin + 2], F32, isOutput=False)
    sc = nc.declare_dram_parameter("sc", [1, 11], F32, isOutput=False)
    out = nc.declare_dram_parameter("o", [tok, FSH], F32, isOutput=True)

    with tile.TileContext(nc) as tc:
        with (
            tc.tile_pool(name="const", bufs=1) as const,
            tc.tile_pool(name="wtpool", bufs=1) as wtpool,
            tc.tile_pool(name="wip", bufs=4) as wip,
            tc.tile_pool(name="wfp", bufs=6) as wfp,
            tc.tile_pool(name="hfp", bufs=3) as hfp,
            tc.tile_pool(name="xb", bufs=3) as xbp,
            tc.tile_pool(name="xt", bufs=2) as xtp,
            tc.tile_pool(name="osb", bufs=3) as osbp,
            tc.tile_pool(name="psw", bufs=4, space="PSUM") as psw,
            tc.tile_pool(name="psx", bufs=4, space="PSUM") as psx,
        ):
            # ---- setup: scalars, identity, band matrices -------------------
            ident = const.tile([128, 128], BF16)
            make_identity(nc, ident[:])

            sc_sb = const.tile([1, 11], F32)
            nc.sync.dma_start(out=sc_sb[:], in_=sc[:])

            ones_r = const.tile([1, 128], F32)
            nc.vector.memset(ones_r[:], 1.0)

            # broadcast the 11 scalars to all 128 partitions via a k=1 matmul
            ps_b = psw.tile([128, 11], F32, tag="pw")
            nc.tensor.matmul(ps_b[:], ones_r[:], sc_sb[:], start=True, stop=True)
            scv = const.tile([128, 11], F32)
            nc.vector.tensor_copy(out=scv[:], in_=ps_b[:])

            # ctr = conv_w[h,1,1] + sigmoid(sk_wt[h])
            sig = const.tile([128, 1], F32)
            nc.scalar.activation(sig[:], scv[:, 10:11],
                                 mybir.ActivationFunctionType.Sigmoid)
            ctr = const.tile([128, 1], F32)
            nc.vector.tensor_tensor(out=ctr[:], in0=sig[:], in1=scv[:, 4:5],
                                    op=mybir.AluOpType.add)

            # band matrices B_dc[k, o] = cw[h, k-o, dc] (k-o in {0,1,2});
            # the dc=1 center diagonal also carries the sigmoid residual.
            masks = []
            for d in range(3):
                m = const.tile([128, 128], F32, tag=f"mask{d}")
                nc.gpsimd.memset(m[:], 0.0)
                nc.gpsimd.affine_select(
                    out=m[:], in_=m[:],
                    compare_op=mybir.AluOpType.not_equal,
                    fill=1.0, base=-d, channel_multiplier=1,
                    pattern=[[-1, 128]],
                )
                masks.append(m)
            b_bf = []
            for dc in range(3):
                bf_ = const.tile([128, 128], F32, tag=f"bf_{dc}")
                nc.vector.tensor_scalar(bf_[:], masks[0][:], scv[:, dc:dc + 1],
                                        None, mybir.AluOpType.mult)
                mid = ctr if dc == 1 else scv[:, 3 + dc:4 + dc]
                nc.vector.scalar_tensor_tensor(
                    out=bf_[:], in0=masks[1][:], scalar=mid, in1=bf_[:],
                    op0=mybir.AluOpType.mult, op1=mybir.AluOpType.add)
                nc.vector.scalar_tensor_tensor(
                    out=bf_[:], in0=masks[2][:], scalar=scv[:, 6 + dc:7 + dc],
                    in1=bf_[:],
                    op0=mybir.AluOpType.mult, op1=mybir.AluOpType.add)
                bb = const.tile([128, 128], BF16, tag=f"bb_{dc}")
                nc.vector.tensor_copy(out=bb[:], in_=bf_[:])
                b_bf.append(bb)

            # halo matrices H_dc [2, 128]: out row 127 takes its dr=1/dr=2
            # taps from halo rows 0/1, and out row 126 its dr=2 tap from halo
            # row 0.  Built as outer products (v.T @ onehot) since engine APs
            # cannot start at a nonzero partition.
            onehot = const.tile([1, 128], F32)
            nc.vector.memset(onehot[:], 0.0)
            nc.vector.memset(onehot[:, 127:128], 1.0)
            onehot6 = const.tile([1, 128], F32)
            nc.vector.memset(onehot6[:], 0.0)
            nc.vector.memset(onehot6[:, 126:127], 1.0)
            sig0 = const.tile([1, 1], F32)
            nc.scalar.activation(sig0[:], sc_sb[:, 10:11],
                                 mybir.ActivationFunctionType.Sigmoid)
            ctr0 = const.tile([1, 1], F32)
            nc.vector.tensor_tensor(out=ctr0[:], in0=sig0[:], in1=sc_sb[:, 4:5],
                                    op=mybir.AluOpType.add)
            # H6 [6, 128]: j = row*3+dc over the 2 halo rows x 3 col shifts.
            # col 127: row0 -> cw[1,dc] (ctr at dc=1), row1 -> cw[2,dc];
            # col 126: row0 -> cw[2,dc].
            v1 = const.tile([1, 6], F32)
            nc.vector.tensor_copy(out=v1[:, 0:1], in_=sc_sb[:, 3:4])
            nc.vector.tensor_copy(out=v1[:, 1:2], in_=ctr0[:])
            nc.vector.tensor_copy(out=v1[:, 2:3], in_=sc_sb[:, 5:6])
            nc.vector.tensor_copy(out=v1[:, 3:6], in_=sc_sb[:, 6:9])
            v2 = const.tile([1, 6], F32)
            nc.vector.memset(v2[:], 0.0)
            nc.vector.tensor_copy(out=v2[:, 0:3], in_=sc_sb[:, 6:9])
            ph = psw.tile([6, 128], F32, tag="pw")
            nc.tensor.matmul(ph[:], v1[:], onehot[:], start=True, stop=False)
            nc.tensor.matmul(ph[:], v2[:], onehot6[:], start=False, stop=True)
            h6 = const.tile([6, 128], BF16)
            nc.vector.tensor_copy(out=h6[:], in_=ph[:])

            wt = wtpool.tile([128, n_k, FSH], BF16)        # W_i^T, fin-major

            t_reps = repeat if "rep_t" in probe else 1
            m_reps = repeat if "rep_t" not in probe else 1

            for rep in range(t_reps):
                # ---- phase T: weight transform + transpose, s-outer -------
                for s in range(n_strip):
                    wiws = []
                    for w in range(n_win):
                        wf = wfp.tile([128, 514], F32, tag="wf")
                        nc.sync.dma_start(
                            out=wf[:],
                            in_=wh[128 * w:128 * w + 128,
                                   512 * s:512 * s + 514])
                        hf = hfp.tile([6, 512], F32, tag="hf")
                        nc.sync.dma_start(
                            out=hf[:],
                            in_=bass.AP(
                                wh.tensor if hasattr(wh, "tensor") else wh,
                                (128 * w + 128) * (fin + 2) + 512 * s,
                                [[fin + 2, 2], [1, 3], [1, 512]]))
                        wrow = wfp.tile([128, 514], BF16, tag="wrow")
                        hrow = hfp.tile([6, 512], BF16, tag="hrow")
                        if w % 2 == 0:
                            nc.vector.tensor_copy(out=wrow[:], in_=wf[:])
                            nc.vector.tensor_copy(out=hrow[:], in_=hf[:])
                        else:
                            nc.scalar.copy(out=wrow[:], in_=wf[:])
                            nc.scalar.copy(out=hrow[:], in_=hf[:])
                        pw = psw.tile([128, 512], F32, tag="pw")
                        for dc in range(3):
                            nc.tensor.matmul(
                                pw[:], b_bf[dc][:], wrow[:, dc:dc + 512],
                                start=(dc == 0), stop=False)
                        nc.tensor.matmul(pw[:], h6[:], hrow[:],
                                         start=False, stop=True)
                        # PSUM -> SBUF with bias add, cast to bf16
                        wiw = wip.tile([128, 512], BF16, tag="wi")
                        if w % 2 == 0:
                            nc.scalar.add(wiw[:], pw[:], scv[:, 9:10])
                        else:
                            nc.vector.tensor_scalar(
                                wiw[:], pw[:], scv[:, 9:10], None,
                                mybir.AluOpType.add)
                        wiws.append(wiw)
                    # transpose W_i strips into W_i^T, two windows per bank
                    for wp in range(0, n_win, 2):
                        pt = psx.tile([128, 1024], BF16, tag="px")
                        for dw in range(2):
                            for j in range(4):
                                nc.tensor.transpose(
                                    pt[:, 512 * dw + 128 * j:
                                       512 * dw + 128 * j + 128],
                                    wiws[wp + dw][:, 128 * j:128 * j + 128],
                                    ident[:])
                        dst = wt[:, 4 * s:4 * s + 4,
                                 128 * wp:128 * wp + 256]
                        srcv = pt[:].rearrange("p (a b c) -> p b a c",
                                               a=2, b=4, c=128)
                        if wp == 0:
                            nc.scalar.copy(out=dst, in_=srcv)
                        else:
                            nc.vector.tensor_copy(out=dst, in_=srcv)

            for rep in range(m_reps):
                # ---- phase M: main matmul ---------------------------------
                for t in range(n_tblk):
                    xb = xbp.tile([128, fin], BF16, tag="xb")
                    if "no_inp_dma" not in probe:
                        xf = xbp.tile([128, fin], F32, tag="xf")
                        nc.sync.dma_start(out=xf[:],
                                          in_=inp[128 * t:128 * t + 128, :])
                        nc.vector.tensor_copy(out=xb[:, :fin // 2],
                                              in_=xf[:, :fin // 2])
                        nc.scalar.copy(out=xb[:, fin // 2:],
                                       in_=xf[:, fin // 2:])
                    xt = xtp.tile([128, n_k, 128], BF16, tag="xt")
                    for ko in range(0 if "no_tr" in probe else n_k // 8):
                        px = psx.tile([128, 1024], BF16, tag="px")
                        for ki in range(8):
                            k = 8 * ko + ki
                            nc.tensor.transpose(
                                px[:, 128 * ki:128 * ki + 128],
                                xb[:, 128 * k:128 * k + 128],
                                ident[:])
                        dst = xt[:, 8 * ko:8 * ko + 8, :]
                        if ko % 2 == 0:
                            nc.vector.tensor_copy(out=dst, in_=px[:])
                        else:
                            nc.scalar.copy(out=dst, in_=px[:])
                    po = psw.tile([128, FSH], F32, tag="pw")
                    if "no_mm" in probe:
                        nc.vector.memset(po[:], 0.0)
                    else:
                        for k in range(n_k):
                            nc.tensor.matmul(po[:], xt[:, k, :], wt[:, k, :],
                                             start=(k == 0),
                                             stop=(k == n_k - 1))
                    ob = osbp.tile([128, FSH], F32, tag="ob")
                    if t % 2 == 0:
                        nc.scalar.copy(out=ob[:], in_=po[:])
                    else:
                        nc.vector.tensor_copy(out=ob[:], in_=po[:])
                    nc.sync.dma_start(out=out[128 * t:128 * t + 128, :],
                                      in_=ob[:])

    nc.compile()
    return nc


def shard_inputs(inp, W, conv_w, conv_b, sk_wt, fin=FIN):
    """Build the 8 per-core input maps (W fout-shard with conv halo)."""
    tok = inp.size // fin
    inp2 = np.ascontiguousarray(inp.reshape(tok, fin), dtype=np.float32)
    W = np.asarray(W, dtype=np.float32)
    hsz = W.shape[0] // NUM_HEADS  # rows per head
    in_maps = []
    for c in range(NCORES):
        gr0 = c * FSH
        h = (gr0 // hsz) % NUM_HEADS
        whal = np.zeros((FSH + 2, fin + 2), dtype=np.float32)
        lo = max(gr0 - 1, h * hsz)
        hi = min(gr0 + FSH + 1, (h + 1) * hsz)
        whal[lo - (gr0 - 1):hi - (gr0 - 1), 1:fin + 1] = W[lo:hi, :fin]
        scal = np.zeros((1, 11), dtype=np.float32)
        scal[0, :9] = np.asarray(conv_w, dtype=np.float32)[h].reshape(9)
        scal[0, 9] = np.float32(np.asarray(conv_b)[h])
        scal[0, 10] = np.float32(np.asarray(sk_wt)[h].reshape(()))
        in_maps.append({"inp": inp2, "wh": whal, "sc": scal})
    return in_maps


_PROGRAM_CACHE = {}


def _get_program(tok, fin, repeat=1):
    key = (tok, fin, repeat)
    if key not in _PROGRAM_CACHE:
        _PROGRAM_CACHE[key] = build_program(tok, fin, repeat)
    return _PROGRAM_CACHE[key]


def kernel(inp, W, conv_w, conv_b, sk_wt):
    nc = _get_program(TOK, FIN)
    in_maps = shard_inputs(inp, W, conv_w, conv_b, sk_wt)
    res = run_bass_kernel_spmd(nc, in_maps, list(range(NCORES)))
    shards = [res.results[c]["o"].reshape(2, TOK // 2, FSH)
              for c in range(NCORES)]
    return np.ascontiguousarray(
        np.concatenate(shards, axis=-1).astype(np.float32))



# revision 2
# speedup vs baseline: 1.1184x; 1.1184x over previous
"""Trainium2 Bass kernel for the FCBlock weight-transform + matmul problem.

Math (per reference):
    W_i = per-head 3x3 conv over W.reshape(4, 1024, 4096) + conv_b
          + sigmoid(sk_wt) * W            (per-head scalars)
    out  = inp @ W_i.T                    (inp: [2, 2048, 4096])

Strategy: tensor-parallel shard of W along fout across 8 NeuronCores
(512 rows each; the host pre-slices W with 1-row conv halo, zero-padded
at head boundaries, and pre-casts W and inp to bf16).  On each core:
  - build the 3x3 conv as tiny banded matrices (from conv_w/conv_b/sk_wt,
    broadcast on device) and run the weight transform as PE band-matmuls
    accumulating in PSUM (sigmoid-gated residual folded into the center
    diagonal; bias added during the PSUM->SBUF copy),
  - transpose W_i on the PE (fin onto partitions),
  - stream inp via X-bar DMA-transpose (bf16) directly into fin-major
    layout, and run the main matmul in bf16 with fp32 PSUM accumulation.
Output is sharded on fout; the host concatenates.
"""

import numpy as np

import concourse.bass as bass
import concourse.mybir as mybir
import concourse.tile as tile
from concourse import bacc
from concourse.bass_utils import run_bass_kernel_spmd
from concourse.masks import make_identity

F32 = mybir.dt.float32
BF16 = mybir.dt.bfloat16

NCORES = 8
NUM_HEADS = 4
TOK = 4096          # 2 * 2048 tokens
FIN = 4096
FOUT = 4096
FSH = FOUT // NCORES  # 512 fout rows per core
SUP = 512           # token superblock (one transpose-DMA each)


def build_program(tok=TOK, fin=FIN):
    """Build the per-core SPMD program.

    tok/fin are parameters so a mini variant can be compiled quickly for
    validation; the graded path always uses the full sizes.
    """
    assert tok % SUP == 0 and fin % 512 == 0
    n_sup = tok // SUP           # 512-token superblocks
    n_strip = fin // 512         # 512-col fin strips
    n_k = fin // 128             # 128-deep contraction blocks
    n_win = FSH // 128           # 4 fout row windows per core

    nc = bacc.Bacc(None, target_bir_lowering=False)

    xb = nc.declare_dram_parameter("xb", [tok, fin], BF16, isOutput=False)
    wh = nc.declare_dram_parameter("wh", [FSH + 2, fin + 2], BF16,
                                   isOutput=False)
    sc = nc.declare_dram_parameter("sc", [1, 11], F32, isOutput=False)
    out = nc.declare_dram_parameter("o", [tok, FSH], F32, isOutput=True)

    with tile.TileContext(nc) as tc:
        with (
            tc.tile_pool(name="const", bufs=1) as const,
            tc.tile_pool(name="wtpool", bufs=1) as wtpool,
            tc.tile_pool(name="wip", bufs=4) as wip,
            tc.tile_pool(name="wfp", bufs=4) as wfp,
            tc.tile_pool(name="hfp", bufs=4) as hfp,
            tc.tile_pool(name="xt", bufs=2) as xtp,
            tc.tile_pool(name="osb", bufs=4) as osbp,
            tc.tile_pool(name="psw", bufs=2, space="PSUM") as psw,
            tc.tile_pool(name="pst", bufs=2, space="PSUM") as pst,
            tc.tile_pool(name="psm", bufs=4, space="PSUM") as psm,
        ):
            # ---- setup: scalars, identity, band matrices -------------------
            ident = const.tile([128, 128], BF16)
            make_identity(nc, ident[:])

            sc_sb = const.tile([1, 11], F32)
            nc.sync.dma_start(out=sc_sb[:], in_=sc[:])

            ones_r = const.tile([1, 128], F32)
            nc.vector.memset(ones_r[:], 1.0)

            # broadcast the 11 scalars to all 128 partitions via a k=1 matmul
            ps_b = psw.tile([128, 11], F32, tag="pw")
            nc.tensor.matmul(ps_b[:], ones_r[:], sc_sb[:], start=True,
                             stop=True)
            scv = const.tile([128, 11], F32)
            nc.vector.tensor_copy(out=scv[:], in_=ps_b[:])

            # ctr = conv_w[h,1,1] + sigmoid(sk_wt[h])
            sig = const.tile([128, 1], F32)
            nc.scalar.activation(sig[:], scv[:, 10:11],
                                 mybir.ActivationFunctionType.Sigmoid)
            ctr = const.tile([128, 1], F32)
            nc.vector.tensor_tensor(out=ctr[:], in0=sig[:], in1=scv[:, 4:5],
                                    op=mybir.AluOpType.add)

            # band matrices B_dc[k, o] = cw[h, k-o, dc] (k-o in {0,1,2});
            # the dc=1 center diagonal also carries the sigmoid residual.
            masks = []
            for d in range(3):
                m = const.tile([128, 128], F32, tag=f"mask{d}")
                nc.gpsimd.memset(m[:], 0.0)
                nc.gpsimd.affine_select(
                    out=m[:], in_=m[:],
                    compare_op=mybir.AluOpType.not_equal,
                    fill=1.0, base=-d, channel_multiplier=1,
                    pattern=[[-1, 128]],
                )
                masks.append(m)
            b_bf = []
            for dc in range(3):
                bf_ = const.tile([128, 128], F32, tag=f"bf_{dc}")
                nc.vector.tensor_scalar(bf_[:], masks[0][:], scv[:, dc:dc + 1],
                                        None, mybir.AluOpType.mult)
                mid = ctr if dc == 1 else scv[:, 3 + dc:4 + dc]
                nc.vector.scalar_tensor_tensor(
                    out=bf_[:], in0=masks[1][:], scalar=mid, in1=bf_[:],
                    op0=mybir.AluOpType.mult, op1=mybir.AluOpType.add)
                nc.vector.scalar_tensor_tensor(
                    out=bf_[:], in0=masks[2][:], scalar=scv[:, 6 + dc:7 + dc],
                    in1=bf_[:],
                    op0=mybir.AluOpType.mult, op1=mybir.AluOpType.add)
                bb = const.tile([128, 128], BF16, tag=f"bb_{dc}")
                nc.vector.tensor_copy(out=bb[:], in_=bf_[:])
                b_bf.append(bb)

            # halo matrices H_dc [2, 128]: out row 127 takes its dr=1/dr=2
            # taps from halo rows 0/1, and out row 126 its dr=2 tap from halo
            # row 0.  Built as outer products (v.T @ onehot) since engine APs
            # cannot start at a nonzero partition.
            onehot = const.tile([1, 128], F32)
            nc.vector.memset(onehot[:], 0.0)
            nc.vector.memset(onehot[:, 127:128], 1.0)
            onehot6 = const.tile([1, 128], F32)
            nc.vector.memset(onehot6[:], 0.0)
            nc.vector.memset(onehot6[:, 126:127], 1.0)
            sig0 = const.tile([1, 1], F32)
            nc.scalar.activation(sig0[:], sc_sb[:, 10:11],
                                 mybir.ActivationFunctionType.Sigmoid)
            ctr0 = const.tile([1, 1], F32)
            nc.vector.tensor_tensor(out=ctr0[:], in0=sig0[:], in1=sc_sb[:, 4:5],
                                    op=mybir.AluOpType.add)
            # H6 [6, 128]: j = row*3+dc over the 2 halo rows x 3 col shifts.
            # col 127: row0 -> cw[1,dc] (ctr at dc=1), row1 -> cw[2,dc];
            # col 126: row0 -> cw[2,dc].
            v1 = const.tile([1, 6], F32)
            nc.vector.tensor_copy(out=v1[:, 0:1], in_=sc_sb[:, 3:4])
            nc.vector.tensor_copy(out=v1[:, 1:2], in_=ctr0[:])
            nc.vector.tensor_copy(out=v1[:, 2:3], in_=sc_sb[:, 5:6])
            nc.vector.tensor_copy(out=v1[:, 3:6], in_=sc_sb[:, 6:9])
            v2 = const.tile([1, 6], F32)
            nc.vector.memset(v2[:], 0.0)
            nc.vector.tensor_copy(out=v2[:, 0:3], in_=sc_sb[:, 6:9])
            ph = psw.tile([6, 128], F32, tag="pw")
            nc.tensor.matmul(ph[:], v1[:], onehot[:], start=True, stop=False)
            nc.tensor.matmul(ph[:], v2[:], onehot6[:], start=False, stop=True)
            h6 = const.tile([6, 128], BF16)
            nc.vector.tensor_copy(out=h6[:], in_=ph[:])

            wt = wtpool.tile([128, n_k, FSH], BF16)        # W_i^T, fin-major

            # ---- phase T: weight transform + transpose, s-outer -----------
            for s in range(n_strip):
                wiws = []
                for w in range(n_win):
                    wrow = wfp.tile([128, 514], BF16, tag="wrow")
                    nc.scalar.dma_start(
                        out=wrow[:],
                        in_=wh[128 * w:128 * w + 128,
                               512 * s:512 * s + 514])
                    hrow = hfp.tile([6, 512], BF16, tag="hrow")
                    nc.scalar.dma_start(
                        out=hrow[:],
                        in_=bass.AP(
                            wh.tensor if hasattr(wh, "tensor") else wh,
                            (128 * w + 128) * (fin + 2) + 512 * s,
                            [[fin + 2, 2], [1, 3], [1, 512]]))
                    pw = psw.tile([128, 512], F32, tag="pw")
                    for dc in range(3):
                        nc.tensor.matmul(
                            pw[:], b_bf[dc][:], wrow[:, dc:dc + 512],
                            start=(dc == 0), stop=False)
                    nc.tensor.matmul(pw[:], h6[:], hrow[:],
                                     start=False, stop=True)
                    # PSUM -> SBUF with bias add, cast to bf16
                    wiw = wip.tile([128, 512], BF16, tag="wi")
                    if w % 2 == 0:
                        nc.scalar.add(wiw[:], pw[:], scv[:, 9:10])
                    else:
                        nc.vector.tensor_scalar(
                            wiw[:], pw[:], scv[:, 9:10], None,
                            mybir.AluOpType.add)
                    wiws.append(wiw)
                # transpose W_i strips into W_i^T, two windows per bank
                for wp in range(0, n_win, 2):
                    pt = pst.tile([128, 1024], BF16, tag="px")
                    for dw in range(2):
                        for j in range(4):
                            nc.tensor.transpose(
                                pt[:, 512 * dw + 128 * j:
                                   512 * dw + 128 * j + 128],
                                wiws[wp + dw][:, 128 * j:128 * j + 128],
                                ident[:])
                    dst = wt[:, 4 * s:4 * s + 4,
                             128 * wp:128 * wp + 256]
                    srcv = pt[:].rearrange("p (a b c) -> p b a c",
                                           a=2, b=4, c=128)
                    if wp == 0:
                        nc.scalar.copy(out=dst, in_=srcv)
                    else:
                        nc.vector.tensor_copy(out=dst, in_=srcv)

            # ---- phase M: main matmul ---------------------------------
            for t in range(n_sup):
                xt = xtp.tile([128, n_k, SUP], BF16, tag="xt")
                nc.sync.dma_start(out=xt[:],
                                  in_=xb[SUP * t:SUP * t + SUP, :],
                                  transpose=True)
                for m in range(SUP // 128):
                    po = psm.tile([128, FSH], F32, tag="po")
                    for k in range(n_k):
                        nc.tensor.matmul(po[:],
                                         xt[:, k, 128 * m:128 * m + 128],
                                         wt[:, k, :],
                                         start=(k == 0),
                                         stop=(k == n_k - 1))
                    ob = osbp.tile([128, FSH], F32, tag="ob")
                    if m % 2 == 0:
                        nc.scalar.copy(out=ob[:], in_=po[:])
                    else:
                        nc.vector.tensor_copy(out=ob[:], in_=po[:])
                    row0 = SUP * t + 128 * m
                    nc.gpsimd.dma_start(out=out[row0:row0 + 128, :],
                                        in_=ob[:])

    nc.compile()
    return nc


def shard_inputs(inp, W, conv_w, conv_b, sk_wt, fin=FIN):
    """Build the 8 per-core input maps (W fout-shard with conv halo)."""
    bf = mybir.dt.np(BF16)
    tok = inp.size // fin
    xb = np.ascontiguousarray(
        inp.reshape(tok, fin)).astype(np.float32).astype(bf)
    W = np.asarray(W, dtype=np.float32)
    hsz = W.shape[0] // NUM_HEADS  # rows per head
    in_maps = []
    for c in range(NCORES):
        gr0 = c * FSH
        h = (gr0 // hsz) % NUM_HEADS
        whal = np.zeros((FSH + 2, fin + 2), dtype=np.float32)
        lo = max(gr0 - 1, h * hsz)
        hi = min(gr0 + FSH + 1, (h + 1) * hsz)
        whal[lo - (gr0 - 1):hi - (gr0 - 1), 1:fin + 1] = W[lo:hi, :fin]
        scal = np.zeros((1, 11), dtype=np.float32)
        scal[0, :9] = np.asarray(conv_w, dtype=np.float32)[h].reshape(9)
        scal[0, 9] = np.float32(np.asarray(conv_b)[h])
        scal[0, 10] = np.float32(np.asarray(sk_wt)[h].reshape(()))
        in_maps.append({"xb": xb, "wh": whal.astype(bf), "sc": scal})
    return in_maps


_PROGRAM_CACHE = {}


def _get_program(tok, fin):
    key = (tok, fin)
    if key not in _PROGRAM_CACHE:
        _PROGRAM_CACHE[key] = build_program(tok, fin)
    return _PROGRAM_CACHE[key]


def kernel(inp, W, conv_w, conv_b, sk_wt):
    nc = _get_program(TOK, FIN)
    in_maps = shard_inputs(inp, W, conv_w, conv_b, sk_wt)
    res = run_bass_kernel_spmd(nc, in_maps, list(range(NCORES)))
    shards = [res.results[c]["o"].reshape(2, TOK // 2, FSH)
              for c in range(NCORES)]
    return np.ascontiguousarray(
        np.concatenate(shards, axis=-1).astype(np.float32))


# revision 3
# speedup vs baseline: 1.2679x; 1.1336x over previous
"""Trainium2 Bass kernel for the FCBlock weight-transform + matmul problem.

Math (per reference):
    W_i = per-head 3x3 conv over W.reshape(4, 1024, 4096) + conv_b
          + sigmoid(sk_wt) * W            (per-head scalars)
    out  = inp @ W_i.T                    (inp: [2, 2048, 4096])

Strategy: tensor-parallel shard of W along fout across 8 NeuronCores
(512 rows each; the host pre-slices W with 1-row conv halo zero-padded
at head boundaries, packs it into the SBUF staging layout, and
pre-casts W and inp to bf16).  On each core:
  - stage the whole W shard in SBUF (8 big DMAs, issued ahead of the
    input transposes on the same HWDGE ring so they are not starved),
  - build the 3x3 conv as tiny banded matrices (from conv_w/conv_b/sk_wt
    broadcast on device) and run the weight transform as PE band-matmuls
    accumulating in PSUM (sigmoid-gated residual folded into the center
    diagonal; bias added during the PSUM->SBUF copy),
  - transpose W_i on the PE (fin onto partitions),
  - stream inp via X-bar DMA-transpose (bf16) directly into fin-major
    layout, and run the main matmul in bf16 with fp32 PSUM accumulation.
Output is sharded on fout; the host concatenates.
"""

import numpy as np

import concourse.bass as bass
import concourse.mybir as mybir
import concourse.tile as tile
from concourse import bacc
from concourse.bass_utils import run_bass_kernel_spmd
from concourse.masks import make_identity

F32 = mybir.dt.float32
BF16 = mybir.dt.bfloat16

NCORES = 8
NUM_HEADS = 4
TOK = 4096          # 2 * 2048 tokens
FIN = 4096
FOUT = 4096
FSH = FOUT // NCORES  # 512 fout rows per core
SUP = 512           # token superblock (one transpose-DMA each)


def build_program(tok=TOK, fin=FIN):
    """Build the per-core SPMD program.

    tok/fin are parameters so a mini variant can be compiled quickly for
    validation; the graded path always uses the full sizes.
    """
    assert tok % SUP == 0 and fin % 512 == 0
    n_sup = tok // SUP           # 512-token superblocks
    n_strip = fin // 512         # 512-col fin strips
    n_k = fin // 128             # 128-deep contraction blocks
    n_win = FSH // 128           # 4 fout row windows per core
    n_w = n_strip * n_win        # staged transform windows

    nc = bacc.Bacc(None, target_bir_lowering=False)

    xb = nc.declare_dram_parameter("xb", [tok, fin], BF16, isOutput=False)
    whs = nc.declare_dram_parameter("whs", [128, n_w, 514], BF16,
                                    isOutput=False)
    hhs = nc.declare_dram_parameter("hhs", [6, n_w, 512], BF16,
                                    isOutput=False)
    sc = nc.declare_dram_parameter("sc", [1, 11], F32, isOutput=False)
    out = nc.declare_dram_parameter("o", [tok, FSH], F32, isOutput=True)

    with tile.TileContext(nc) as tc:
        with (
            tc.tile_pool(name="const", bufs=1) as const,
            tc.tile_pool(name="wtpool", bufs=1) as wtpool,
            tc.tile_pool(name="wip", bufs=4) as wip,
            tc.tile_pool(name="xt", bufs=2) as xtp,
            tc.tile_pool(name="osb", bufs=4) as osbp,
            tc.tile_pool(name="psw", bufs=2, space="PSUM") as psw,
            tc.tile_pool(name="pst", bufs=2, space="PSUM") as pst,
            tc.tile_pool(name="psm", bufs=4, space="PSUM") as psm,
        ):
            # ---- input staging: sc + W slab first on the sync ring --------
            sc_sb = const.tile([1, 11], F32)
            nc.sync.dma_start(out=sc_sb[:], in_=sc[:])

            wst = const.tile([128, n_w, 514], BF16, tag="wst")
            hst = const.tile([6, n_w, 512], BF16, tag="hst")
            n_chunk = max(1, n_w // 8)
            for g in range(0, n_w, n_chunk):
                nc.sync.dma_start(out=wst[:, g:g + n_chunk, :],
                                  in_=whs[:, g:g + n_chunk, :])
                nc.sync.dma_start(out=hst[:, g:g + n_chunk, :],
                                  in_=hhs[:, g:g + n_chunk, :])

            # ---- setup: scalars, identity, band matrices -------------------
            ident = const.tile([128, 128], BF16)
            make_identity(nc, ident[:])

            ones_r = const.tile([1, 128], F32)
            nc.vector.memset(ones_r[:], 1.0)

            # broadcast the 11 scalars to all 128 partitions via a k=1 matmul
            ps_b = psw.tile([128, 11], F32, tag="pw")
            nc.tensor.matmul(ps_b[:], ones_r[:], sc_sb[:], start=True,
                             stop=True)
            scv = const.tile([128, 11], F32)
            nc.vector.tensor_copy(out=scv[:], in_=ps_b[:])

            # ctr = conv_w[h,1,1] + sigmoid(sk_wt[h])
            sig = const.tile([128, 1], F32)
            nc.scalar.activation(sig[:], scv[:, 10:11],
                                 mybir.ActivationFunctionType.Sigmoid)
            ctr = const.tile([128, 1], F32)
            nc.vector.tensor_tensor(out=ctr[:], in0=sig[:], in1=scv[:, 4:5],
                                    op=mybir.AluOpType.add)

            # band matrices B_dc[k, o] = cw[h, k-o, dc] (k-o in {0,1,2});
            # the dc=1 center diagonal also carries the sigmoid residual.
            masks = []
            for d in range(3):
                m = const.tile([128, 128], F32, tag=f"mask{d}")
                nc.gpsimd.memset(m[:], 0.0)
                nc.gpsimd.affine_select(
                    out=m[:], in_=m[:],
                    compare_op=mybir.AluOpType.not_equal,
                    fill=1.0, base=-d, channel_multiplier=1,
                    pattern=[[-1, 128]],
                )
                masks.append(m)
            b_bf = []
            for dc in range(3):
                bf_ = const.tile([128, 128], F32, tag=f"bf_{dc}")
                nc.vector.tensor_scalar(bf_[:], masks[0][:], scv[:, dc:dc + 1],
                                        None, mybir.AluOpType.mult)
                mid = ctr if dc == 1 else scv[:, 3 + dc:4 + dc]
                nc.vector.scalar_tensor_tensor(
                    out=bf_[:], in0=masks[1][:], scalar=mid, in1=bf_[:],
                    op0=mybir.AluOpType.mult, op1=mybir.AluOpType.add)
                nc.vector.scalar_tensor_tensor(
                    out=bf_[:], in0=masks[2][:], scalar=scv[:, 6 + dc:7 + dc],
                    in1=bf_[:],
                    op0=mybir.AluOpType.mult, op1=mybir.AluOpType.add)
                bb = const.tile([128, 128], BF16, tag=f"bb_{dc}")
                nc.vector.tensor_copy(out=bb[:], in_=bf_[:])
                b_bf.append(bb)

            # halo matrices H_dc [2, 128]: out row 127 takes its dr=1/dr=2
            # taps from halo rows 0/1, and out row 126 its dr=2 tap from halo
            # row 0.  Built as outer products (v.T @ onehot) since engine APs
            # cannot start at a nonzero partition.
            onehot = const.tile([1, 128], F32)
            nc.vector.memset(onehot[:], 0.0)
            nc.vector.memset(onehot[:, 127:128], 1.0)
            onehot6 = const.tile([1, 128], F32)
            nc.vector.memset(onehot6[:], 0.0)
            nc.vector.memset(onehot6[:, 126:127], 1.0)
            sig0 = const.tile([1, 1], F32)
            nc.scalar.activation(sig0[:], sc_sb[:, 10:11],
                                 mybir.ActivationFunctionType.Sigmoid)
            ctr0 = const.tile([1, 1], F32)
            nc.vector.tensor_tensor(out=ctr0[:], in0=sig0[:], in1=sc_sb[:, 4:5],
                                    op=mybir.AluOpType.add)
            # H6 [6, 128]: j = row*3+dc over the 2 halo rows x 3 col shifts.
            # col 127: row0 -> cw[1,dc] (ctr at dc=1), row1 -> cw[2,dc];
            # col 126: row0 -> cw[2,dc].
            v1 = const.tile([1, 6], F32)
            nc.vector.tensor_copy(out=v1[:, 0:1], in_=sc_sb[:, 3:4])
            nc.vector.tensor_copy(out=v1[:, 1:2], in_=ctr0[:])
            nc.vector.tensor_copy(out=v1[:, 2:3], in_=sc_sb[:, 5:6])
            nc.vector.tensor_copy(out=v1[:, 3:6], in_=sc_sb[:, 6:9])
            v2 = const.tile([1, 6], F32)
            nc.vector.memset(v2[:], 0.0)
            nc.vector.tensor_copy(out=v2[:, 0:3], in_=sc_sb[:, 6:9])
            ph = psw.tile([6, 128], F32, tag="pw")
            nc.tensor.matmul(ph[:], v1[:], onehot[:], start=True, stop=False)
            nc.tensor.matmul(ph[:], v2[:], onehot6[:], start=False, stop=True)
            h6 = const.tile([6, 128], BF16)
            nc.vector.tensor_copy(out=h6[:], in_=ph[:])

            wt = wtpool.tile([128, n_k, FSH], BF16)        # W_i^T, fin-major

            # ---- phase T: weight transform + transpose, s-outer -----------
            for s in range(n_strip):
                wiws = []
                for w in range(n_win):
                    i = n_win * s + w
                    pw = psw.tile([128, 512], F32, tag="pw")
                    for dc in range(3):
                        nc.tensor.matmul(
                            pw[:], b_bf[dc][:], wst[:, i, dc:dc + 512],
                            start=(dc == 0), stop=False)
                    nc.tensor.matmul(pw[:], h6[:], hst[:, i, :],
                                     start=False, stop=True)
                    # PSUM -> SBUF with bias add, cast to bf16
                    wiw = wip.tile([128, 512], BF16, tag="wi")
                    if w % 2 == 0:
                        nc.scalar.add(wiw[:], pw[:], scv[:, 9:10])
                    else:
                        nc.vector.tensor_scalar(
                            wiw[:], pw[:], scv[:, 9:10], None,
                            mybir.AluOpType.add)
                    wiws.append(wiw)
                # transpose W_i strips into W_i^T, two windows per bank
                for wp in range(0, n_win, 2):
                    pt = pst.tile([128, 1024], BF16, tag="px")
                    for dw in range(2):
                        for j in range(4):
                            nc.tensor.transpose(
                                pt[:, 512 * dw + 128 * j:
                                   512 * dw + 128 * j + 128],
                                wiws[wp + dw][:, 128 * j:128 * j + 128],
                                ident[:])
                    dst = wt[:, 4 * s:4 * s + 4,
                             128 * wp:128 * wp + 256]
                    srcv = pt[:].rearrange("p (a b c) -> p b a c",
                                           a=2, b=4, c=128)
                    if wp == 0:
                        nc.scalar.copy(out=dst, in_=srcv)
                    else:
                        nc.vector.tensor_copy(out=dst, in_=srcv)

            # ---- phase M: main matmul ---------------------------------
            for t in range(n_sup):
                xt = xtp.tile([128, n_k, SUP], BF16, tag="xt")
                nc.sync.dma_start(out=xt[:],
                                  in_=xb[SUP * t:SUP * t + SUP, :],
                                  transpose=True)
                for m in range(SUP // 128):
                    po = psm.tile([128, FSH], F32, tag="po")
                    for k in range(n_k):
                        nc.tensor.matmul(po[:],
                                         xt[:, k, 128 * m:128 * m + 128],
                                         wt[:, k, :],
                                         start=(k == 0),
                                         stop=(k == n_k - 1))
                    ob = osbp.tile([128, FSH], F32, tag="ob")
                    if m % 2 == 0:
                        nc.scalar.copy(out=ob[:], in_=po[:])
                    else:
                        nc.vector.tensor_copy(out=ob[:], in_=po[:])
                    row0 = SUP * t + 128 * m
                    nc.gpsimd.dma_start(out=out[row0:row0 + 128, :],
                                        in_=ob[:])

    nc.compile()
    return nc


def shard_inputs(inp, W, conv_w, conv_b, sk_wt, fin=FIN):
    """Build the 8 per-core input maps (W fout-shard with conv halo,
    packed into the on-device staging layout)."""
    bf = mybir.dt.np(BF16)
    tok = inp.size // fin
    xb = np.ascontiguousarray(
        inp.reshape(tok, fin)).astype(np.float32).astype(bf)
    W = np.asarray(W, dtype=np.float32)
    hsz = W.shape[0] // NUM_HEADS  # rows per head
    n_strip = fin // 512
    n_win = FSH // 128
    n_w = n_strip * n_win
    in_maps = []
    for c in range(NCORES):
        gr0 = c * FSH
        h = (gr0 // hsz) % NUM_HEADS
        whal = np.zeros((FSH + 2, fin + 2), dtype=np.float32)
        lo = max(gr0 - 1, h * hsz)
        hi = min(gr0 + FSH + 1, (h + 1) * hsz)
        whal[lo - (gr0 - 1):hi - (gr0 - 1), 1:fin + 1] = W[lo:hi, :fin]
        whs = np.empty((128, n_w, 514), dtype=np.float32)
        hhs = np.empty((6, n_w, 512), dtype=np.float32)
        for s in range(n_strip):
            for w in range(n_win):
                i = n_win * s + w
                whs[:, i, :] = whal[128 * w:128 * w + 128,
                                    512 * s:512 * s + 514]
                for a in range(2):
                    for b in range(3):
                        hhs[3 * a + b, i, :] = whal[128 * w + 128 + a,
                                                    512 * s + b:
                                                    512 * s + b + 512]
        scal = np.zeros((1, 11), dtype=np.float32)
        scal[0, :9] = np.asarray(conv_w, dtype=np.float32)[h].reshape(9)
        scal[0, 9] = np.float32(np.asarray(conv_b)[h])
        scal[0, 10] = np.float32(np.asarray(sk_wt)[h].reshape(()))
        in_maps.append({"xb": xb, "whs": whs.astype(bf),
                        "hhs": hhs.astype(bf), "sc": scal})
    return in_maps


_PROGRAM_CACHE = {}


def _get_program(tok, fin):
    key = (tok, fin)
    if key not in _PROGRAM_CACHE:
        _PROGRAM_CACHE[key] = build_program(tok, fin)
    return _PROGRAM_CACHE[key]


def kernel(inp, W, conv_w, conv_b, sk_wt):
    nc = _get_program(TOK, FIN)
    in_maps = shard_inputs(inp, W, conv_w, conv_b, sk_wt)
    res = run_bass_kernel_spmd(nc, in_maps, list(range(NCORES)))
    shards = [res.results[c]["o"].reshape(2, TOK // 2, FSH)
              for c in range(NCORES)]
    return np.ascontiguousarray(
        np.concatenate(shards, axis=-1).astype(np.float32))


# revision 4
# speedup vs baseline: 1.2852x; 1.0137x over previous
"""Trainium2 Bass kernel for the FCBlock weight-transform + matmul problem.

Math (per reference):
    W_i = per-head 3x3 conv over W.reshape(4, 1024, 4096) + conv_b
          + sigmoid(sk_wt) * W            (per-head scalars)
    out  = inp @ W_i.T                    (inp: [2, 2048, 4096])

Strategy: tensor-parallel shard of W along fout across 8 NeuronCores
(512 rows each).  The host pre-slices W^T with the conv halo (zero-padded
at head boundaries and fin edges), packs it into the SBUF staging layout,
builds the tiny banded conv matrices from conv_w^T/sigmoid(sk_wt), and
pre-casts everything to bf16.  On each core:
  - stage the whole W^T shard in SBUF (big DMAs issued ahead of the input
    transposes on the same HWDGE ring so they are not starved),
  - run the weight transform as PE band-matmuls accumulating in PSUM,
    which directly yields W_i^T (fin on partitions) - no transposes;
    the conv bias is added during the PSUM->SBUF copy,
  - stream inp via X-bar DMA-transpose (bf16) directly into fin-major
    layout, and run the main matmul in bf16 with fp32 PSUM accumulation.
Output is sharded on fout; the host concatenates.
"""

import numpy as np

import concourse.bass as bass
import concourse.mybir as mybir
import concourse.tile as tile
from concourse import bacc
from concourse.bass_utils import run_bass_kernel_spmd

F32 = mybir.dt.float32
BF16 = mybir.dt.bfloat16

NCORES = 8
NUM_HEADS = 4
TOK = 4096          # 2 * 2048 tokens
FIN = 4096
FOUT = 4096
FSH = FOUT // NCORES  # 512 fout rows per core
SUP = 512           # token superblock (one transpose-DMA each)


def build_program(tok=TOK, fin=FIN):
    """Build the per-core SPMD program.

    tok/fin are parameters so a mini variant can be compiled quickly for
    validation; the graded path always uses the full sizes.
    """
    assert tok % SUP == 0 and fin % 128 == 0
    n_sup = tok // SUP           # 512-token superblocks
    n_k = fin // 128             # 128-deep contraction blocks / T windows

    nc = bacc.Bacc(None, target_bir_lowering=False)

    xb = nc.declare_dram_parameter("xb", [tok, fin], BF16, isOutput=False)
    wts = nc.declare_dram_parameter("wts", [128, n_k, FSH + 2], BF16,
                                    isOutput=False)
    hts = nc.declare_dram_parameter("hts", [6, n_k, FSH], BF16,
                                    isOutput=False)
    cbnd = nc.declare_dram_parameter("cbnd", [128, 4, 128], BF16,
                                     isOutput=False)
    cb = nc.declare_dram_parameter("cb", [128, 1], F32, isOutput=False)
    out = nc.declare_dram_parameter("o", [tok, FSH], F32, isOutput=True)

    with tile.TileContext(nc) as tc:
        with (
            tc.tile_pool(name="const", bufs=1) as const,
            tc.tile_pool(name="wtpool", bufs=1) as wtpool,
            tc.tile_pool(name="xt", bufs=2) as xtp,
            tc.tile_pool(name="osb", bufs=4) as osbp,
            tc.tile_pool(name="ps", bufs=6, space="PSUM") as ps,
        ):
            # ---- input staging: consts + W^T slab first on the sync ring --
            cbnd_sb = const.tile([128, 4, 128], BF16)
            nc.sync.dma_start(out=cbnd_sb[:], in_=cbnd[:])
            cb_sb = const.tile([128, 1], F32)
            nc.sync.dma_start(out=cb_sb[:], in_=cb[:])

            wst = const.tile([128, n_k, FSH + 2], BF16, tag="wst")
            hst = const.tile([6, n_k, FSH], BF16, tag="hst")
            n_chunk = max(1, n_k // 8)
            for g in range(0, n_k, n_chunk):
                nc.sync.dma_start(out=wst[:, g:g + n_chunk, :],
                                  in_=wts[:, g:g + n_chunk, :])
                nc.sync.dma_start(out=hst[:, g:g + n_chunk, :],
                                  in_=hts[:, g:g + n_chunk, :])

            wt = wtpool.tile([128, n_k, FSH], BF16)        # W_i^T, fin-major

            # ---- phase T: weight transform straight into W_i^T ------------
            for i in range(n_k):
                pw = ps.tile([128, FSH], F32, tag="ps")
                for a in range(3):
                    nc.tensor.matmul(
                        pw[:], cbnd_sb[:, a, :], wst[:, i, a:a + FSH],
                        start=(a == 0), stop=False)
                nc.tensor.matmul(pw[:], cbnd_sb[0:6, 3, :], hst[:, i, :],
                                 start=False, stop=True)
                # PSUM -> SBUF with conv-bias add, cast to bf16
                if i % 2 == 0:
                    nc.scalar.add(wt[:, i, :], pw[:], cb_sb[:, 0:1])
                else:
                    nc.vector.tensor_scalar(
                        wt[:, i, :], pw[:], cb_sb[:, 0:1], None,
                        mybir.AluOpType.add)

            # ---- phase M: main matmul ---------------------------------
            for t in range(n_sup):
                xt = xtp.tile([128, n_k, SUP], BF16, tag="xt")
                nc.sync.dma_start(out=xt[:],
                                  in_=xb[SUP * t:SUP * t + SUP, :],
                                  transpose=True)
                for m in range(SUP // 128):
                    po = ps.tile([128, FSH], F32, tag="ps")
                    for k in range(n_k):
                        nc.tensor.matmul(po[:],
                                         xt[:, k, 128 * m:128 * m + 128],
                                         wt[:, k, :],
                                         start=(k == 0),
                                         stop=(k == n_k - 1))
                    ob = osbp.tile([128, FSH], F32, tag="ob")
                    if m % 2 == 0:
                        nc.scalar.copy(out=ob[:], in_=po[:])
                    else:
                        nc.vector.tensor_copy(out=ob[:], in_=po[:])
                    row0 = SUP * t + 128 * m
                    nc.gpsimd.dma_start(out=out[row0:row0 + 128, :],
                                        in_=ob[:])

    nc.compile()
    return nc


def shard_inputs(inp, W, conv_w, conv_b, sk_wt, fin=FIN):
    """Build the 8 per-core input maps: W^T fout-shard with conv halo,
    packed into the on-device staging layout, plus host-built band
    matrices (conv taps transposed, sigmoid residual folded in)."""
    bf = mybir.dt.np(BF16)
    tok = inp.size // fin
    xb = np.ascontiguousarray(
        inp.reshape(tok, fin)).astype(np.float32).astype(bf)
    W = np.asarray(W, dtype=np.float32)
    conv_w = np.asarray(conv_w, dtype=np.float32)
    hsz = W.shape[0] // NUM_HEADS  # rows per head
    n_k = fin // 128
    in_maps = []
    for c in range(NCORES):
        gr0 = c * FSH
        h = (gr0 // hsz) % NUM_HEADS
        # whal[R, C] = W[gr0-1+R, C-1], zero outside the head / fin range
        whal = np.zeros((FSH + 2, fin + 2), dtype=np.float32)
        lo = max(gr0 - 1, h * hsz)
        hi = min(gr0 + FSH + 1, (h + 1) * hsz)
        whal[lo - (gr0 - 1):hi - (gr0 - 1), 1:fin + 1] = W[lo:hi, :fin]
        # staged W^T: wts[k, i, c] = whal[c, 128i + k]
        wtslab = np.ascontiguousarray(whal.T)          # [fin+2, FSH+2]
        wts = np.ascontiguousarray(
            wtslab[:n_k * 128].reshape(n_k, 128, FSH + 2)
            .transpose(1, 0, 2))                       # [128, n_k, FSH+2]
        hts = np.empty((6, n_k, FSH), dtype=np.float32)
        for a in range(2):
            for b in range(3):
                hts[3 * a + b] = wtslab[128 + a:128 * n_k + 128 + a:128,
                                        b:b + FSH]
        # band matrices (conv taps transposed); sigmoid residual on the
        # (a=1, d=1) diagonal; halo matrix in cbnd[:, 3, :]
        cwt = conv_w[h].reshape(3, 3).T
        sig = float(1.0 / (1.0 + np.exp(-np.float64(
            np.asarray(sk_wt, dtype=np.float32)[h].reshape(())))))
        cbnd = np.zeros((128, 4, 128), dtype=np.float32)
        for a in range(3):
            for d in range(3):
                cbnd[:, a, :] += np.eye(128, k=-d, dtype=np.float32) \
                    * cwt[d, a]
        cbnd[:, 1, :] += np.eye(128, k=-1, dtype=np.float32) * sig
        h6 = np.zeros((6, 128), dtype=np.float32)
        for b in range(3):
            h6[b, 127] = cwt[1, b]
            h6[3 + b, 127] = cwt[2, b]
            h6[b, 126] = cwt[2, b]
        h6[1, 127] += sig
        cbnd[0:6, 3, :] = h6
        cbv = np.full((128, 1), np.float32(np.asarray(conv_b)[h]),
                      dtype=np.float32)
        in_maps.append({"xb": xb, "wts": wts.astype(bf),
                        "hts": hts.astype(bf),
                        "cbnd": cbnd.astype(bf), "cb": cbv})
    return in_maps


_PROGRAM_CACHE = {}


def _get_program(tok, fin):
    key = (tok, fin)
    if key not in _PROGRAM_CACHE:
        _PROGRAM_CACHE[key] = build_program(tok, fin)
    return _PROGRAM_CACHE[key]


def kernel(inp, W, conv_w, conv_b, sk_wt):
    nc = _get_program(TOK, FIN)
    in_maps = shard_inputs(inp, W, conv_w, conv_b, sk_wt)
    res = run_bass_kernel_spmd(nc, in_maps, list(range(NCORES)))
    shards = [res.results[c]["o"].reshape(2, TOK // 2, FSH)
              for c in range(NCORES)]
    return np.ascontiguousarray(
        np.concatenate(shards, axis=-1).astype(np.float32))


# revision 7
# speedup vs baseline: 1.2972x; 1.0093x over previous
"""Trainium2 Bass kernel for the FCBlock weight-transform + matmul problem.

Math (per reference):
    W_i = per-head 3x3 conv over W.reshape(4, 1024, 4096) + conv_b
          + sigmoid(sk_wt) * W            (per-head scalars)
    out  = inp @ W_i.T                    (inp: [2, 2048, 4096])

Strategy: tensor-parallel shard of W along fout across 8 NeuronCores
(512 rows each).  The host pre-slices W^T with the conv halo (zero-padded
at head boundaries and fin edges), packs it into the SBUF staging layout,
builds the tiny banded conv matrices from conv_w^T/sigmoid(sk_wt), and
pre-casts everything to bf16.  On each core:
  - stage the whole W^T shard in SBUF (big DMAs issued ahead of the input
    transposes on the same HWDGE ring so they are not starved),
  - run the weight transform as PE band-matmuls accumulating in PSUM,
    which directly yields W_i^T (fin on partitions) - no transposes;
    the conv bias is added during the PSUM->SBUF copy,
  - stream inp via X-bar DMA-transpose (bf16) directly into fin-major
    layout, and run the main matmul in bf16 with fp32 PSUM accumulation.
Output is sharded on fout; the host concatenates.
"""

import numpy as np

import concourse.bass as bass
import concourse.mybir as mybir
import concourse.tile as tile
from concourse import bacc
from concourse.bass_utils import run_bass_kernel_spmd

F32 = mybir.dt.float32
BF16 = mybir.dt.bfloat16

NCORES = 8
NUM_HEADS = 4
TOK = 4096          # 2 * 2048 tokens
FIN = 4096
FOUT = 4096
FSH = FOUT // NCORES  # 512 fout rows per core
SUP = 512           # token superblock (one transpose-DMA each)


def build_program(tok=TOK, fin=FIN):
    """Build the per-core SPMD program.

    tok/fin are parameters so a mini variant can be compiled quickly for
    validation; the graded path always uses the full sizes.
    """
    assert tok % SUP == 0 and fin % 128 == 0
    n_sup = tok // SUP           # 512-token superblocks
    n_k = fin // 128             # 128-deep contraction blocks / T windows

    nc = bacc.Bacc(None, target_bir_lowering=False)

    xb = nc.declare_dram_parameter("xb", [tok, fin], BF16, isOutput=False)
    wts = nc.declare_dram_parameter("wts", [128, n_k, FSH + 2], BF16,
                                    isOutput=False)
    hts = nc.declare_dram_parameter("hts", [6, n_k, FSH], BF16,
                                    isOutput=False)
    cbnd = nc.declare_dram_parameter("cbnd", [128, 4, 128], BF16,
                                     isOutput=False)
    cb = nc.declare_dram_parameter("cb", [128, 1], F32, isOutput=False)
    out = nc.declare_dram_parameter("o", [tok, FSH], F32, isOutput=True)

    with tile.TileContext(nc) as tc:
        with (
            tc.tile_pool(name="const", bufs=1) as const,
            tc.tile_pool(name="wtpool", bufs=1) as wtpool,
            tc.tile_pool(name="xt", bufs=2) as xtp,
            tc.tile_pool(name="osb", bufs=6) as osbp,
            tc.tile_pool(name="ps", bufs=8, space="PSUM") as ps,
        ):
            # ---- input staging: consts + W^T slab first on the sync ring --
            cbnd_sb = const.tile([128, 4, 128], BF16)
            nc.sync.dma_start(out=cbnd_sb[:], in_=cbnd[:])
            cb_sb = const.tile([128, 1], F32)
            nc.sync.dma_start(out=cb_sb[:], in_=cb[:])

            wst = const.tile([128, n_k, FSH + 2], BF16, tag="wst")
            hst = const.tile([6, n_k, FSH], BF16, tag="hst")
            nc.sync.dma_start(out=hst[:], in_=hts[:])
            # graded chunks: tiny first so phase T starts ASAP, then big
            g = 0
            for sz in (1, 1, 2, 4, 8, 8, 8, 8, 8, 8, 8, 8):
                if g >= n_k:
                    break
                sz = min(sz, n_k - g)
                nc.sync.dma_start(out=wst[:, g:g + sz, :],
                                  in_=wts[:, g:g + sz, :])
                g += sz

            wt = wtpool.tile([128, n_k, FSH], BF16)        # W_i^T, fin-major

            # ---- phase T: weight transform straight into W_i^T ------------
            for i in range(n_k):
                pw = ps.tile([128, FSH], F32, tag="ps")
                for a in range(3):
                    nc.tensor.matmul(
                        pw[:], cbnd_sb[:, a, :], wst[:, i, a:a + FSH],
                        start=(a == 0), stop=False)
                nc.tensor.matmul(pw[:], cbnd_sb[0:6, 3, :], hst[:, i, :],
                                 start=False, stop=True)
                # PSUM -> SBUF with conv-bias add, cast to bf16
                if i % 2 == 0:
                    nc.scalar.add(wt[:, i, :], pw[:], cb_sb[:, 0:1])
                else:
                    nc.vector.tensor_scalar(
                        wt[:, i, :], pw[:], cb_sb[:, 0:1], None,
                        mybir.AluOpType.add)

            # ---- phase M: main matmul ---------------------------------
            for t in range(n_sup):
                xt = xtp.tile([128, n_k, SUP], BF16, tag="xt")
                nc.sync.dma_start(out=xt[:],
                                  in_=xb[SUP * t:SUP * t + SUP, :],
                                  transpose=True)
                for m in range(SUP // 128):
                    po = ps.tile([128, FSH], F32, tag="ps")
                    for k in range(n_k):
                        nc.tensor.matmul(po[:],
                                         xt[:, k, 128 * m:128 * m + 128],
                                         wt[:, k, :],
                                         start=(k == 0),
                                         stop=(k == n_k - 1))
                    ob = osbp.tile([128, FSH], F32, tag="ob")
                    if m % 2 == 0:
                        nc.scalar.copy(out=ob[:], in_=po[:])
                    else:
                        nc.vector.tensor_copy(out=ob[:], in_=po[:])
                    row0 = SUP * t + 128 * m
                    nc.scalar.dma_start(out=out[row0:row0 + 128, :],
                                        in_=ob[:])

    nc.compile()
    return nc


def shard_inputs(inp, W, conv_w, conv_b, sk_wt, fin=FIN):
    """Build the 8 per-core input maps: W^T fout-shard with conv halo,
    packed into the on-device staging layout, plus host-built band
    matrices (conv taps transposed, sigmoid residual folded in)."""
    bf = mybir.dt.np(BF16)
    tok = inp.size // fin
    xb = np.ascontiguousarray(
        inp.reshape(tok, fin)).astype(np.float32).astype(bf)
    W = np.asarray(W, dtype=np.float32)
    conv_w = np.asarray(conv_w, dtype=np.float32)
    hsz = W.shape[0] // NUM_HEADS  # rows per head
    n_k = fin // 128
    in_maps = []
    for c in range(NCORES):
        gr0 = c * FSH
        h = (gr0 // hsz) % NUM_HEADS
        # whal[R, C] = W[gr0-1+R, C-1], zero outside the head / fin range
        whal = np.zeros((FSH + 2, fin + 2), dtype=np.float32)
        lo = max(gr0 - 1, h * hsz)
        hi = min(gr0 + FSH + 1, (h + 1) * hsz)
        whal[lo - (gr0 - 1):hi - (gr0 - 1), 1:fin + 1] = W[lo:hi, :fin]
        # staged W^T: wts[k, i, c] = whal[c, 128i + k]
        wtslab = np.ascontiguousarray(whal.T)          # [fin+2, FSH+2]
        wts = np.ascontiguousarray(
            wtslab[:n_k * 128].reshape(n_k, 128, FSH + 2)
            .transpose(1, 0, 2))                       # [128, n_k, FSH+2]
        hts = np.empty((6, n_k, FSH), dtype=np.float32)
        for a in range(2):
            for b in range(3):
                hts[3 * a + b] = wtslab[128 + a:128 * n_k + 128 + a:128,
                                        b:b + FSH]
        # band matrices (conv taps transposed); sigmoid residual on the
        # (a=1, d=1) diagonal; halo matrix in cbnd[:, 3, :]
        cwt = conv_w[h].reshape(3, 3).T
        sig = float(1.0 / (1.0 + np.exp(-np.float64(
            np.asarray(sk_wt, dtype=np.float32)[h].reshape(())))))
        cbnd = np.zeros((128, 4, 128), dtype=np.float32)
        for a in range(3):
            for d in range(3):
                cbnd[:, a, :] += np.eye(128, k=-d, dtype=np.float32) \
                    * cwt[d, a]
        cbnd[:, 1, :] += np.eye(128, k=-1, dtype=np.float32) * sig
        h6 = np.zeros((6, 128), dtype=np.float32)
        for b in range(3):
            h6[b, 127] = cwt[1, b]
            h6[3 + b, 127] = cwt[2, b]
            h6[b, 126] = cwt[2, b]
        h6[1, 127] += sig
        cbnd[0:6, 3, :] = h6
        cbv = np.full((128, 1), np.float32(np.asarray(conv_b)[h]),
                      dtype=np.float32)
        in_maps.append({"xb": xb, "wts": wts.astype(bf),
                        "hts": hts.astype(bf),
                        "cbnd": cbnd.astype(bf), "cb": cbv})
    return in_maps


_PROGRAM_CACHE = {}


def _get_program(tok, fin):
    key = (tok, fin)
    if key not in _PROGRAM_CACHE:
        _PROGRAM_CACHE[key] = build_program(tok, fin)
    return _PROGRAM_CACHE[key]


def kernel(inp, W, conv_w, conv_b, sk_wt):
    nc = _get_program(TOK, FIN)
    in_maps = shard_inputs(inp, W, conv_w, conv_b, sk_wt)
    res = run_bass_kernel_spmd(nc, in_maps, list(range(NCORES)))
    shards = [res.results[c]["o"].reshape(2, TOK // 2, FSH)
              for c in range(NCORES)]
    return np.ascontiguousarray(
        np.concatenate(shards, axis=-1).astype(np.float32))


# revision 10
# speedup vs baseline: 1.3445x; 1.0365x over previous
"""Trainium2 Bass kernel for the FCBlock weight-transform + matmul problem.

Math (per reference):
    W_i = per-head 3x3 conv over W.reshape(4, 1024, 4096) + conv_b
          + sigmoid(sk_wt) * W            (per-head scalars)
    out  = inp @ W_i.T                    (inp: [2, 2048, 4096])

Strategy: tensor-parallel shard of W along fout across 8 NeuronCores
(512 rows each).  The host pre-slices W^T with the conv halo (zero-padded
at head boundaries and fin edges), packs it into the SBUF staging layout,
builds the tiny banded conv matrices from conv_w^T/sigmoid(sk_wt), and
pre-casts everything to bf16.  On each core:
  - stage the whole W^T shard in SBUF (big DMAs issued ahead of the input
    transposes on the same HWDGE ring so they are not starved),
  - run the weight transform as PE band-matmuls accumulating in PSUM,
    which directly yields W_i^T (fin on partitions) - no transposes;
    the conv bias is added during the PSUM->SBUF copy,
  - stream inp via X-bar DMA-transpose (bf16) directly into fin-major
    layout, and run the main matmul in bf16 with fp32 PSUM accumulation.
Output is sharded on fout; the host concatenates.
"""

import numpy as np

import concourse.bass as bass
import concourse.mybir as mybir
import concourse.tile as tile
from concourse import bacc
from concourse.bass_utils import run_bass_kernel_spmd

F32 = mybir.dt.float32
BF16 = mybir.dt.bfloat16

NCORES = 8
NUM_HEADS = 4
TOK = 4096          # 2 * 2048 tokens
FIN = 4096
FOUT = 4096
FSH = FOUT // NCORES  # 512 fout rows per core
SUP = 512           # token superblock (one transpose-DMA each)


def build_program(tok=TOK, fin=FIN):
    """Build the per-core SPMD program.

    tok/fin are parameters so a mini variant can be compiled quickly for
    validation; the graded path always uses the full sizes.
    """
    assert tok % SUP == 0 and fin % 128 == 0
    n_sup = tok // SUP           # 512-token superblocks
    n_k = fin // 128             # 128-deep contraction blocks / T windows

    nc = bacc.Bacc(None, target_bir_lowering=False)

    xb = nc.declare_dram_parameter("xb", [tok, fin], BF16, isOutput=False)
    wts = nc.declare_dram_parameter("wts", [128, n_k, FSH + 2], BF16,
                                    isOutput=False)
    hts = nc.declare_dram_parameter("hts", [6, n_k, FSH], BF16,
                                    isOutput=False)
    cbnd = nc.declare_dram_parameter("cbnd", [128, 4, 128], BF16,
                                     isOutput=False)
    cb = nc.declare_dram_parameter("cb", [128, 1], F32, isOutput=False)
    out = nc.declare_dram_parameter("o", [tok, FSH], F32, isOutput=True)

    with tile.TileContext(nc) as tc:
        with (
            tc.tile_pool(name="const", bufs=1) as const,
            tc.tile_pool(name="wtpool", bufs=1) as wtpool,
            tc.tile_pool(name="xt", bufs=2) as xtp,
            tc.tile_pool(name="osb", bufs=6) as osbp,
            tc.tile_pool(name="ps", bufs=8, space="PSUM") as ps,
        ):
            # ---- input staging: W^T slab + consts first on the sync ring --
            wst = const.tile([128, n_k, FSH + 2], BF16, tag="wst")
            hst = const.tile([6, n_k, FSH], BF16, tag="hst")
            cbnd_sb = const.tile([128, 4, 128], BF16)
            cb_sb = const.tile([128, 1], F32)
            n_chunk = max(1, n_k // 4)
            nc.sync.dma_start(out=wst[:, 0:n_chunk, :],
                              in_=wts[:, 0:n_chunk, :])
            nc.sync.dma_start(out=cbnd_sb[:], in_=cbnd[:])
            nc.sync.dma_start(out=cb_sb[:], in_=cb[:])
            nc.sync.dma_start(out=hst[:], in_=hts[:])
            for g in range(n_chunk, n_k, n_chunk):
                nc.sync.dma_start(out=wst[:, g:g + n_chunk, :],
                                  in_=wts[:, g:g + n_chunk, :])

            wt = wtpool.tile([128, n_k, FSH], BF16)        # W_i^T, fin-major

            # ---- phase T: weight transform straight into W_i^T ------------
            # window quads with a-outer ordering: amortizes stationary
            # switches (the [6,...] halo tiles break LDWEIGHTS pull-ahead)
            nq = 4
            for q in range(0, n_k, nq):
                pws = [ps.tile([128, FSH], F32, tag="ps", name=f"pw{q}_{j}")
                       for j in range(min(nq, n_k - q))]
                for a in range(3):
                    for j, pw in enumerate(pws):
                        nc.tensor.matmul(
                            pw[:], cbnd_sb[:, a, :],
                            wst[:, q + j, a:a + FSH],
                            start=(a == 0), stop=False)
                for j, pw in enumerate(pws):
                    nc.tensor.matmul(pw[:], cbnd_sb[0:6, 3, :],
                                     hst[:, q + j, :],
                                     start=False, stop=True)
                for j, pw in enumerate(pws):
                    i = q + j
                    # PSUM -> SBUF with conv-bias add, cast to bf16
                    if j % 2 == 0:
                        nc.scalar.add(wt[:, i, :], pw[:], cb_sb[:, 0:1])
                    else:
                        nc.vector.tensor_scalar(
                            wt[:, i, :], pw[:], cb_sb[:, 0:1], None,
                            mybir.AluOpType.add)

            # ---- phase M: main matmul ---------------------------------
            for t in range(n_sup):
                xt = xtp.tile([128, n_k, SUP], BF16, tag="xt")
                nc.sync.dma_start(out=xt[:],
                                  in_=xb[SUP * t:SUP * t + SUP, :],
                                  transpose=True)
                for m in range(SUP // 128):
                    po = ps.tile([128, FSH], F32, tag="ps")
                    for k in range(n_k):
                        nc.tensor.matmul(po[:],
                                         xt[:, k, 128 * m:128 * m + 128],
                                         wt[:, k, :],
                                         start=(k == 0),
                                         stop=(k == n_k - 1))
                    ob = osbp.tile([128, FSH], F32, tag="ob")
                    if m % 2 == 0:
                        nc.scalar.copy(out=ob[:], in_=po[:])
                    else:
                        nc.vector.tensor_copy(out=ob[:], in_=po[:])
                    row0 = SUP * t + 128 * m
                    nc.scalar.dma_start(out=out[row0:row0 + 128, :],
                                        in_=ob[:])

    nc.compile()
    return nc


def shard_inputs(inp, W, conv_w, conv_b, sk_wt, fin=FIN):
    """Build the 8 per-core input maps: W^T fout-shard with conv halo,
    packed into the on-device staging layout, plus host-built band
    matrices (conv taps transposed, sigmoid residual folded in)."""
    bf = mybir.dt.np(BF16)
    tok = inp.size // fin
    xb = np.ascontiguousarray(
        inp.reshape(tok, fin)).astype(np.float32).astype(bf)
    W = np.asarray(W, dtype=np.float32)
    conv_w = np.asarray(conv_w, dtype=np.float32)
    hsz = W.shape[0] // NUM_HEADS  # rows per head
    n_k = fin // 128
    in_maps = []
    for c in range(NCORES):
        gr0 = c * FSH
        h = (gr0 // hsz) % NUM_HEADS
        # whal[R, C] = W[gr0-1+R, C-1], zero outside the head / fin range
        whal = np.zeros((FSH + 2, fin + 2), dtype=np.float32)
        lo = max(gr0 - 1, h * hsz)
        hi = min(gr0 + FSH + 1, (h + 1) * hsz)
        whal[lo - (gr0 - 1):hi - (gr0 - 1), 1:fin + 1] = W[lo:hi, :fin]
        # staged W^T: wts[k, i, c] = whal[c, 128i + k]
        wtslab = np.ascontiguousarray(whal.T)          # [fin+2, FSH+2]
        wts = np.ascontiguousarray(
            wtslab[:n_k * 128].reshape(n_k, 128, FSH + 2)
            .transpose(1, 0, 2))                       # [128, n_k, FSH+2]
        hts = np.empty((6, n_k, FSH), dtype=np.float32)
        for a in range(2):
            for b in range(3):
                hts[3 * a + b] = wtslab[128 + a:128 * n_k + 128 + a:128,
                                        b:b + FSH]
        # band matrices (conv taps transposed); sigmoid residual on the
        # (a=1, d=1) diagonal; halo matrix in cbnd[:, 3, :]
        cwt = conv_w[h].reshape(3, 3).T
        sig = float(1.0 / (1.0 + np.exp(-np.float64(
            np.asarray(sk_wt, dtype=np.float32)[h].reshape(())))))
        cbnd = np.zeros((128, 4, 128), dtype=np.float32)
        for a in range(3):
            for d in range(3):
                cbnd[:, a, :] += np.eye(128, k=-d, dtype=np.float32) \
                    * cwt[d, a]
        cbnd[:, 1, :] += np.eye(128, k=-1, dtype=np.float32) * sig
        h6 = np.zeros((6, 128), dtype=np.float32)
        for b in range(3):
            h6[b, 127] = cwt[1, b]
            h6[3 + b, 127] = cwt[2, b]
            h6[b, 126] = cwt[2, b]
        h6[1, 127] += sig
        cbnd[0:6, 3, :] = h6
        cbv = np.full((128, 1), np.float32(np.asarray(conv_b)[h]),
                      dtype=np.float32)
        in_maps.append({"xb": xb, "wts": wts.astype(bf),
                        "hts": hts.astype(bf),
                        "cbnd": cbnd.astype(bf), "cb": cbv})
    return in_maps


_PROGRAM_CACHE = {}


def _get_program(tok, fin):
    key = (tok, fin)
    if key not in _PROGRAM_CACHE:
        _PROGRAM_CACHE[key] = build_program(tok, fin)
    return _PROGRAM_CACHE[key]


def kernel(inp, W, conv_w, conv_b, sk_wt):
    nc = _get_program(TOK, FIN)
    in_maps = shard_inputs(inp, W, conv_w, conv_b, sk_wt)
    res = run_bass_kernel_spmd(nc, in_maps, list(range(NCORES)))
    shards = [res.results[c]["o"].reshape(2, TOK // 2, FSH)
              for c in range(NCORES)]
    return np.ascontiguousarray(
        np.concatenate(shards, axis=-1).astype(np.float32))
